# revision 1
# baseline (speedup 1.0000x reference)
"""Multi-head causal attention (B=2, S=2048, E=1024, H=16, D=64) on 8 TRN2 cores.

Sharding: core c -> batch b = c // 4, head group g = c % 4 (4 heads each).
Each core computes q/k/v projections + RoPE + causal attention + its rows of
the Wo projection for its (batch, head-group); the host sums the 4 row-parallel
Wo partials per batch (the unshard step of row-parallel output projection).

Device layout notes:
  - x is passed pre-transposed per batch: xT [E, S] so the PE can contract
    over E (partition dim) for the projections.
  - q/k are computed transposed (qT/kT [64, S]) with head-pair fused weights
    so one [128, 512] PSUM tile holds [q_x1; q_x2; k_x1; k_x2] rows, where
    x1/x2 are the RoPE even/odd pair halves (weight columns pre-permuted on
    host so rotate-half applies).
  - scores are computed transposed, sT [k, q] = kT.T @ qT; softmax runs over
    the partition dim via an appended ones-column in the AV matmul (Z row).
    No max-subtraction: scores ~ N(0,1), exp is safe in fp32.
  - AV computes attnT [d, q]; Wo projection contracts head-dim chunks of
    attnT against Wo rows (fp16), accumulating out [s, e] tiles in PSUM.
"""

import sys

if "/opt/trn_rl_repo" not in sys.path:
    sys.path.insert(0, "/opt/trn_rl_repo")

import numpy as np

import concourse.bass as bass
import concourse.tile as tile
from concourse import bacc, mybir
from concourse.bass_utils import run_bass_kernel_spmd

B, S, E, H, D = 2, 2048, 1024, 16, 64
HPC = 4  # heads per core
NCORES = 8
SB = 512  # q/s block width
NSB = S // SB  # 4
KT = 128  # k tile (partition chunk of the sequence)
NKT = S // KT  # 16
ECH = E // 128  # 8 contraction chunks for the projections

f32 = mybir.dt.float32
f16 = mybir.dt.float16
bf16 = mybir.dt.bfloat16

ROPE_BASE = 10000.0


def build_nc(unroll=1):
    nc = bacc.Bacc(
        "TRN2", target_bir_lowering=False, debug=False, enable_asserts=False
    )

    xT_d = nc.dram_tensor("xT", [E, S], f16, kind="ExternalInput")
    wqk_d = nc.dram_tensor("wqk", [E, HPC, 128], f16, kind="ExternalInput")
    wv_d = nc.dram_tensor("wv", [E, HPC * D], f16, kind="ExternalInput")
    wo_d = nc.dram_tensor("wo", [HPC * D, E], f16, kind="ExternalInput")
    cos_d = nc.dram_tensor("cos2", [128, S], f32, kind="ExternalInput")
    sin_d = nc.dram_tensor("sin2", [128, S], f32, kind="ExternalInput")
    mask_d = nc.dram_tensor("maskb", [128, 4, SB], f16, kind="ExternalInput")
    tri_d = nc.dram_tensor("tri", [128, 128], f16, kind="ExternalInput")
    out_d = nc.dram_tensor("out", [S, E], f16, kind="ExternalOutput")

    with tile.TileContext(nc) as tc:
        with (
            tc.tile_pool(name="const", bufs=1) as constp,
            tc.tile_pool(name="qk", bufs=1) as qkp,
            tc.tile_pool(name="vb", bufs=1) as vbp,
            tc.tile_pool(name="at", bufs=1) as atp,
            tc.tile_pool(name="st", bufs=12) as stp,
            tc.tile_pool(name="tmp", bufs=4) as tmpp,
            tc.tile_pool(name="mm", bufs=2, space="PSUM") as mmp,
            tc.tile_pool(name="wps", bufs=2, space="PSUM") as wpsp,
            tc.tile_pool(name="acc", bufs=1, space="PSUM") as accp,
        ):
            # ---- constant tiles (DMAs issued per s-block, in consumption
            # order, so the first projection matmuls start within a few us) --
            xT_ap = xT_d.ap().rearrange("(eo p) s -> eo p s", p=128)
            xT = [
                constp.tile([128, S], f16, tag=f"xT{e}", name=f"xT{e}")
                for e in range(ECH)
            ]
            wqk = constp.tile([128, ECH, HPC, 128], f16, tag="wqk")
            nc.sync.dma_start(
                out=wqk, in_=wqk_d.ap().rearrange("(eo p) h m -> p eo h m", p=128)
            )
            wv = constp.tile([128, ECH, HPC * D], f16, tag="wv")
            nc.sync.dma_start(
                out=wv, in_=wv_d.ap().rearrange("(eo p) m -> p eo m", p=128)
            )
            cos2 = constp.tile([128, S], f32, tag="cos2")
            sin2 = constp.tile([128, S], f32, tag="sin2")
            maskb = constp.tile([128, 4, SB], f16, tag="maskb")
            tri = constp.tile([128, 128], f16, tag="tri")
            wo = constp.tile([128, 2, E], f16, tag="wo")

            def emit_loads(sb):
                cs = slice(sb * SB, (sb + 1) * SB)
                for e in range(ECH):
                    nc.sync.dma_start(out=xT[e][:, cs], in_=xT_ap[e][:, cs])
                nc.sync.dma_start(out=cos2[:, cs], in_=cos_d.ap()[:, cs])
                nc.sync.dma_start(out=sin2[:, cs], in_=sin_d.ap()[:, cs])
                if sb == 0:
                    nc.sync.dma_start(out=maskb, in_=mask_d.ap())
                    nc.sync.dma_start(out=tri, in_=tri_d.ap())
                if sb == 1:
                    nc.sync.dma_start(
                        out=wo, in_=wo_d.ap().rearrange("(c p) e -> p c e", p=128)
                    )

            # qq[p] rows: qT of head 2p on partitions 0-63, head 2p+1 on 64-127
            # (kk[p] likewise) so each head's scores matmul operands share a
            # partition base. psum rows per head: [q_x1; q_x2; k_x1; k_x2].
            qq = [
                qkp.tile([128, S], f16, tag=f"qq{p}", name=f"qq{p}")
                for p in range(2)
            ]
            kk = [
                qkp.tile([128, S], f16, tag=f"kk{p}", name=f"kk{p}")
                for p in range(2)
            ]
            swap_src = [32, 0, 96, 64]

            def emit_qk_proj(sb):
                cs = slice(sb * SB, (sb + 1) * SB)
                for h in range(HPC):
                    p, half = h // 2, (h % 2) * 64
                    ps = mmp.tile([128, SB], f32, tag="mm", name="ps")
                    for e in range(ECH):
                        nc.tensor.matmul(
                            out=ps,
                            lhsT=wqk[:, e, h, :],
                            rhs=xT[e][:, cs],
                            start=(e == 0),
                            stop=(e == ECH - 1),
                        )
                    rs = tmpp.tile([128, SB], f32, tag="rs", name="rs")
                    nc.scalar.copy(out=rs, in_=ps)
                    t1 = tmpp.tile([128, SB], f32, tag="t1", name="t1")
                    t2 = tmpp.tile([128, SB], f32, tag="t2", name="t2")
                    nc.vector.tensor_mul(t1, rs, cos2[:, cs])
                    for g in range(4):
                        # sin2 rows are laid out so in0/in1 share a base
                        # partition (walrus SB+SB constraint)
                        srow = swap_src[g]
                        nc.vector.tensor_mul(
                            t2[g * 32 : (g + 1) * 32, :],
                            rs[srow : srow + 32, :],
                            sin2[srow : srow + 32, cs],
                        )
                    nc.vector.tensor_add(
                        qq[p][half : half + 64, cs], t1[0:64, :], t2[0:64, :]
                    )
                    nc.vector.tensor_add(
                        kk[p][half : half + 64, cs], t1[64:128, :], t2[64:128, :]
                    )

            # v_big free layout per k-chunk: 4 heads x [v_h (64) | one (1)]
            v_big = vbp.tile([128, NKT, HPC * 65], f16, tag="vbig")
            ones_cols = v_big.rearrange("p n (h m) -> p n h m", h=HPC)[
                :, :, :, 64:65
            ]
            nc.vector.memset(ones_cols, 1.0)

            def emit_v_proj(sb):
                for kc in range(4 * sb, 4 * sb + 4):
                    vps = mmp.tile([128, HPC * D], f32, tag="mm", name="vps")
                    for e in range(ECH):
                        nc.tensor.matmul(
                            out=vps,
                            lhsT=xT[e][:, kc * KT : (kc + 1) * KT],
                            rhs=wv[:, e, :],
                            start=(e == 0),
                            stop=(e == ECH - 1),
                        )
                    nc.vector.tensor_copy(
                        out=v_big.rearrange("p n (h m) -> p n h m", h=HPC)[
                            :, kc, :, 0:64
                        ],
                        in_=vps.rearrange("p (h m) -> p h m", h=HPC),
                    )

            # ---- phase C: attention per (q block, head pair) --------------------
            # attnT tiles: at8[c][qb] rows = hd chunk c (2 heads x 64), cols = q
            # Heads 2p / 2p+1 sit at partition bases 0 / 64 of qq[p]/kk[p], so
            # their K=64 scores matmuls land in disjoint PE row groups and run
            # concurrently (row tiling via auto tile_position).
            at8 = {}
            for c in range(2):
                for qb in range(NSB):
                    at8[(c, qb)] = atp.tile(
                        [128, SB], f16, tag=f"at{c}_{qb}", name=f"at{c}_{qb}"
                    )

            def emit_attn(qb):
                qs = slice(qb * SB, (qb + 1) * SB)
                n_k = 4 * (qb + 1)
                for p in range(2):
                    # one wide [128, 1024] PSUM pair-tile per head pair: both
                    # heads' scores live side by side so a single ACT exp
                    # covers them (halves exp instructions and sem hops)
                    av2 = accp.tile([128, 2 * SB], f32, tag="acc", name="av2")
                    # Software pipeline: emit the AV matmul for chunk kt only
                    # LAG steps after its scores matmul, so the PE (strict
                    # in-order queue) never head-of-line blocks on the ACT exp.
                    LAG = 2
                    sts_buf = {}
                    for step in range(n_k + LAG):
                        if step < n_k:
                            kt = step
                            j = kt - 4 * qb
                            kts = slice(kt * KT, (kt + 1) * KT)
                            ps2 = wpsp.tile(
                                [128, 2 * SB], f32, tag="wps", name="ps2"
                            )
                            for i in range(2):
                                half = i * 64
                                nc.tensor.matmul(
                                    out=ps2[:, i * SB : (i + 1) * SB],
                                    lhsT=kk[p][half : half + 64, kts],
                                    rhs=qq[p][half : half + 64, qs],
                                    start=True,
                                    stop=(j < 0),
                                )
                                if j >= 0:
                                    # causal mask: add -240*max(0, r+128j-c)
                                    # (tri.T @ maskb_j); exp(0.125*x) -> 0
                                    nc.tensor.matmul(
                                        out=ps2[:, i * SB : (i + 1) * SB],
                                        lhsT=tri,
                                        rhs=maskb[:, j, :],
                                        start=False,
                                        stop=True,
                                    )
                            st_t = stp.tile(
                                [128, 2 * SB], f16, tag="st", name="st_t"
                            )
                            nc.scalar.activation(
                                out=st_t,
                                in_=ps2,
                                func=mybir.ActivationFunctionType.Exp,
                                scale=0.125,
                            )
                            sts_buf[kt] = st_t
                        if step >= LAG:
                            kt = step - LAG
                            st_t = sts_buf.pop(kt)
                            for i in range(2):
                                h = 2 * p + i
                                nc.tensor.matmul(
                                    out=av2[0:65, i * SB : (i + 1) * SB],
                                    lhsT=v_big[:, kt, h * 65 : (h + 1) * 65],
                                    rhs=st_t[:, i * SB : (i + 1) * SB],
                                    start=(kt == 0),
                                    stop=(kt == n_k - 1),
                                )
                    # normalize: attnT = av[0:64] / Z  (Z = av row 64)
                    for i in range(2):
                        h = 2 * p + i
                        avi = av2[:, i * SB : (i + 1) * SB]
                        r = tmpp.tile([1, SB], f32, tag="r", name="r")
                        nc.vector.reciprocal(out=r, in_=avi[64:65, :])
                        zb = tmpp.tile([64, SB], f32, tag="zb", name="zb")
                        nc.gpsimd.partition_broadcast(zb, r)
                        c, half = h // 2, (h % 2) * 64
                        nc.vector.tensor_mul(
                            at8[(c, qb)][half : half + 64, :], avi[0:64, :], zb
                        )

            # ---- phase D: output projection (row-parallel partial) -------------
            def emit_out_proj(qb):
                for stl in range(4):
                    rows = qb * SB + stl * KT
                    for eb in range(2):
                        pw = mmp.tile([128, SB], f32, tag="mm", name="pw")
                        for c in range(2):
                            nc.tensor.matmul(
                                out=pw,
                                lhsT=at8[(c, qb)][:, stl * KT : (stl + 1) * KT],
                                rhs=wo[:, c, eb * SB : (eb + 1) * SB],
                                start=(c == 0),
                                stop=(c == 1),
                            )
                        ot = stp.tile([128, SB], f16, tag="ot", name="ot", bufs=3)
                        nc.vector.tensor_copy(out=ot, in_=pw)
                        nc.sync.dma_start(
                            out=out_d.ap()[rows : rows + KT, eb * SB : (eb + 1) * SB],
                            in_=ot,
                        )

            # ---- emission schedule: pipeline loads/proj with attention ----------
            # unroll > 1 repeats the whole kernel for overhead-free timing
            for _ in range(unroll):
                emit_loads(0)
                emit_qk_proj(0)
                emit_v_proj(0)
                emit_loads(1)
                emit_qk_proj(1)
                emit_v_proj(1)
                emit_attn(0)
                emit_loads(2)
                emit_qk_proj(2)
                emit_v_proj(2)
                emit_attn(1)
                emit_loads(3)
                emit_qk_proj(3)
                emit_v_proj(3)
                emit_out_proj(0)
                emit_attn(2)
                emit_out_proj(1)
                emit_attn(3)
                emit_out_proj(2)
                emit_out_proj(3)

    nc.compile()
    return nc


def build_in_maps(x, Wq, Wk, Wv, Wo):
    x = np.asarray(x, np.float32)
    Wq = np.asarray(Wq, np.float32)
    Wk = np.asarray(Wk, np.float32)
    Wv = np.asarray(Wv, np.float32)
    Wo = np.asarray(Wo, np.float32)

    # RoPE tables in rotate-half layout ([32] pair-frequencies, duplicated)
    inv = 1.0 / (ROPE_BASE ** (np.arange(0, D, 2, dtype=np.float64) / D))  # [32]
    ang = inv[:, None] * np.arange(S, dtype=np.float64)[None, :]  # [32, S]
    cos_t = np.cos(ang).astype(np.float32)
    sin_t = np.sin(ang).astype(np.float32)
    cos2 = np.concatenate([cos_t, cos_t, cos_t, cos_t], 0)  # [128, S]
    sin2 = np.concatenate([sin_t, -sin_t, sin_t, -sin_t], 0)  # [128, S] (rows at swap-source positions)

    # Causal mask matmul operands: accumulating tri.T @ maskb_j into the
    # scores psum adds -240*max(0, r + 128j - c), which the exp flushes to 0
    # exactly on the masked (k > q) region.
    tt = np.arange(128)[:, None]
    cc = np.arange(SB)[None, :]
    maskb = np.ascontiguousarray(
        np.stack([(cc < tt + j * KT) for j in range(4)], axis=1)
    ).astype(np.float16)  # [128, 4, SB]
    rr = np.arange(128)[None, :]
    tri = (-240.0 * (tt <= rr)).astype(np.float16)  # [t, r]

    # weight column permutation: even pair-elements then odd (rotate-half)
    perm = np.concatenate([np.arange(0, D, 2), np.arange(1, D, 2)])

    in_maps = []
    for core in range(NCORES):
        b, g = core // HPC, core % HPC
        wqk = np.empty((E, HPC, 128), np.float32)
        for i in range(HPC):
            h = g * HPC + i
            wqk[:, i, 0:64] = Wq[:, h * D : (h + 1) * D][:, perm]
            wqk[:, i, 64:128] = Wk[:, h * D : (h + 1) * D][:, perm]
        in_maps.append(
            {
                "xT": np.ascontiguousarray(x[b].T).astype(np.float16),
                "wqk": wqk.astype(np.float16),
                "wv": np.ascontiguousarray(
                    Wv[:, g * HPC * D : (g + 1) * HPC * D]
                ).astype(np.float16),
                "wo": np.ascontiguousarray(
                    Wo[g * HPC * D : (g + 1) * HPC * D, :]
                ).astype(np.float16),
                "cos2": cos2,
                "sin2": sin2,
                "maskb": maskb,
                "tri": tri,
            }
        )
    return in_maps


def gather_output(results):
    outs = [np.asarray(r["out"], np.float32) for r in results]
    return np.stack(
        [outs[0] + outs[1] + outs[2] + outs[3], outs[4] + outs[5] + outs[6] + outs[7]],
        axis=0,
    )


_NC_CACHE = {}


def kernel(x, Wq, Wk, Wv, Wo):
    in_maps = build_in_maps(x, Wq, Wk, Wv, Wo)
    if "nc" not in _NC_CACHE:
        _NC_CACHE["nc"] = build_nc()
    res = run_bass_kernel_spmd(_NC_CACHE["nc"], in_maps, core_ids=list(range(NCORES)))
    return gather_output(res.results)



# revision 28
# speedup vs baseline: 1.0418x; 1.0418x over previous
"""Multi-head causal attention (B=2, S=2048, E=1024, H=16, D=64) on 8 TRN2 cores.

Sharding: core c -> batch b = c // 4, head group g = c % 4 (4 heads each).
Each core computes q/k/v projections + RoPE + causal attention + its rows of
the Wo projection for its (batch, head-group); the host sums the 4 row-parallel
Wo partials per batch.

v2 design (vs v1):
  - Projections produce q/k in [S, D] orientation so RoPE runs on free-dim
    column swaps (3 wide DVE ops per s-block instead of 7 partition-offset
    ones); qT/kT for the scores matmuls are formed by DMA XBAR transposes
    (off the PE critical path).
  - Causal masking is done post-exp with a [128,128] 0/1 multiply on DVE
    (no tri/maskb matmuls on the PE), and the dead q-columns of diagonal
    k-blocks are trimmed from scores/exp/AV.
  - Scores psum is a [128, 1024] two-head tile -> one wide exp per step.
  - av psum is drained by one [128,1024] copy to SBUF; normalization
    (1/Z scale) happens on the SBUF copy, freeing the psum bank early.
  - Out-projection psum is DMA'd to HBM directly as f32 (no copy).
  - Weight/table loads are split per chunk and interleaved in consumption
    order across the two HWDGE queues (SP: x/wqk/tables/out, ACT: wv/wo/
    qkT transposes).
"""

import sys

if "/opt/trn_rl_repo" not in sys.path:
    sys.path.insert(0, "/opt/trn_rl_repo")

import numpy as np

import concourse.bass as bass
import concourse.tile as tile
from concourse import bacc, mybir
from concourse.bass_utils import run_bass_kernel_spmd

B, S, E, H, D = 2, 2048, 1024, 16, 64
HPC = 4  # heads per core
NCORES = 8
SB = 512  # q block width (attention)
NSB = S // SB  # 4
KT = 128  # k tile (partition chunk of the sequence)
PB = 128  # projection s-block
NPB = S // PB  # 16
ECH = E // 128  # 8 contraction chunks for the projections

f32 = mybir.dt.float32
f16 = mybir.dt.float16

ROPE_BASE = 10000.0


def build_nc(unroll=1):
    nc = bacc.Bacc(
        "TRN2", target_bir_lowering=False, debug=False, enable_asserts=False
    )

    xT_d = nc.dram_tensor("xT", [E, S], f16, kind="ExternalInput")
    wqk_d = nc.dram_tensor("wqk", [E, 2 * HPC * D], f16, kind="ExternalInput")
    wv_d = nc.dram_tensor("wv", [E, HPC * D], f16, kind="ExternalInput")
    wo_d = nc.dram_tensor("wo", [HPC * D, E], f16, kind="ExternalInput")
    cos_d = nc.dram_tensor("cosT", [128, NPB, 32], f16, kind="ExternalInput")
    sin_d = nc.dram_tensor("sinT", [128, NPB, 64], f16, kind="ExternalInput")
    mask_d = nc.dram_tensor("maskc", [128, 128], f16, kind="ExternalInput")
    out_d = nc.dram_tensor("out", [S, E], f16, kind="ExternalOutput")

    with tile.TileContext(nc) as tc:
        with (
            tc.tile_pool(name="const", bufs=1) as constp,
            tc.tile_pool(name="rope", bufs=2) as ropep,
            tc.tile_pool(name="st", bufs=4) as stp,
            tc.tile_pool(name="nrm", bufs=2) as nrmp,
            tc.tile_pool(name="mm", bufs=2, space="PSUM") as mmp,
            tc.tile_pool(name="sc", bufs=2, space="PSUM") as scp,
            tc.tile_pool(name="acc", bufs=1, space="PSUM") as accp,
        ):
            # ---- SBUF residents -------------------------------------------
            xT_ap = xT_d.ap().rearrange("(eo p) s -> p eo s", p=128)
            xT = constp.tile([128, ECH, S], f16, tag="xT")
            wqk = constp.tile([128, ECH, 2 * HPC * D], f16, tag="wqk")
            wqk_ap = wqk_d.ap().rearrange("(eo p) m -> p eo m", p=128)
            wv = constp.tile([128, ECH, HPC * D], f16, tag="wv")
            wo = constp.tile([128, 2, E], f16, tag="wo")
            cosT = constp.tile([128, NPB, 32], f16, tag="cosT")
            sinT = constp.tile([128, NPB, 64], f16, tag="sinT")
            maskc = constp.tile([128, 128], f16, tag="maskc")

            # qkT slots: 0,1 = q pairs (heads 01 / 23), 2,3 = k pairs
            qkbig = constp.tile([128, 4, S], f16, tag="qkbig")
            # v per k-chunk: 4 heads x [v_h (64) | one (1)]
            v_big = constp.tile([128, S // KT, HPC * 65], f16, tag="vbig")
            ones_cols = v_big.rearrange("p n (h m) -> p n h m", h=HPC)[
                :, :, :, 64:65
            ]
            nc.vector.memset(ones_cols, 1.0)

            at8 = {}
            for c in range(2):
                for qb in range(NSB):
                    at8[(c, qb)] = constp.tile(
                        [128, SB], f16, tag=f"at{c}_{qb}", name=f"at{c}_{qb}"
                    )

            # ---- DMA emission helpers -------------------------------------
            def emit_load_head():
                # consumption-ordered: first proj matmul needs wqk[e0]+xT[e0].
                # wv goes on the scalar-engine HWDGE queue (which later
                # carries the qkT XBAR transposes and wo).
                nc.sync.dma_start(out=wqk[:, 0:2, :], in_=wqk_ap[:, 0:2, :])
                nc.sync.dma_start(
                    out=xT[:, 0:2, 0:SB], in_=xT_ap[:, 0:2, 0:SB]
                )
                nc.scalar.dma_start(out=wv, in_=wv_d.ap().rearrange(
                    "(eo p) m -> p eo m", p=128))
                nc.sync.dma_start(out=cosT, in_=cos_d.ap())
                nc.sync.dma_start(out=sinT, in_=sin_d.ap())
                nc.sync.dma_start(
                    out=wqk[:, 2:ECH, :], in_=wqk_ap[:, 2:ECH, :]
                )
                nc.sync.dma_start(
                    out=xT[:, 2:ECH, 0:SB], in_=xT_ap[:, 2:ECH, 0:SB]
                )
                nc.sync.dma_start(out=maskc, in_=mask_d.ap())

            def emit_loads(b):
                cs = slice(b * SB, (b + 1) * SB)
                nc.sync.dma_start(out=xT[:, :, cs], in_=xT_ap[:, :, cs])

            def emit_load_wo():
                nc.scalar.dma_start(
                    out=wo, in_=wo_d.ap().rearrange("(c p) e -> p c e", p=128)
                )

            # ---- per-s-block projection + rope + transpose ----------------
            pending_t = []

            def flush_transposes():
                while pending_t:
                    sb, rout = pending_t.pop(0)
                    # one fused XBAR transpose: [128 s, 4*128 f] ->
                    # qkbig[dd, slot, s] (per-slot 128x128 transpose)
                    nc.scalar.dma_start_transpose(
                        out=qkbig[:, :, sb * PB : (sb + 1) * PB],
                        in_=rout,
                    )

            def emit_qk_proj(sb):
                cs = slice(sb * PB, (sb + 1) * PB)
                ps = mmp.tile([128, 512], f32, tag="mm", name="ps")
                for e in range(ECH):
                    nc.tensor.matmul(
                        out=ps,
                        lhsT=xT[:, e, cs],
                        rhs=wqk[:, e, :],
                        start=(e == 0),
                        stop=(e == ECH - 1),
                    )
                # rope: cols = [q(4h) | k(4h)], per head [x1(32) | x2(32)]
                t1 = ropep.tile([128, 512], f16, tag="t1", name="t1")
                t2 = ropep.tile([128, 512], f16, tag="t2", name="t2")
                rout = ropep.tile([128, 512], f16, tag="ro", name="ro", bufs=3)
                ps4 = ps.rearrange("p (g two i) -> p g two i", two=2, i=32)
                t24 = t2.rearrange("p (g two i) -> p g two i", two=2, i=32)
                nc.vector.tensor_mul(
                    t1.rearrange("p (g i) -> p g i", i=32),
                    ps.rearrange("p (g i) -> p g i", i=32),
                    cosT[:, sb, None, :].broadcast_to((128, 16, 32)),
                )
                # o1 part: -x2*sin ; o2 part: +x1*sin
                nc.vector.tensor_mul(
                    t24[:, :, 0:1, :],
                    ps4[:, :, 1:2, :],
                    sinT[:, sb, None, None, 0:32].broadcast_to((128, 8, 1, 32)),
                )
                nc.vector.tensor_mul(
                    t24[:, :, 1:2, :],
                    ps4[:, :, 0:1, :],
                    sinT[:, sb, None, None, 32:64].broadcast_to((128, 8, 1, 32)),
                )
                nc.vector.tensor_add(rout, t1, t2)
                # defer the XBAR transpose so its rope dependency is already
                # satisfied when the scalar queue reaches it (no head-of-line
                # blocking of the exp stream)
                pending_t.append((sb, rout))

            def emit_v_proj(sb):
                pv = mmp.tile([128, 512], f32, tag="mm", name="pv")
                for e in range(ECH):
                    nc.tensor.matmul(
                        out=pv[:, 0 : HPC * D],
                        lhsT=xT[:, e, sb * PB : (sb + 1) * PB],
                        rhs=wv[:, e, :],
                        start=(e == 0),
                        stop=(e == ECH - 1),
                    )
                nc.vector.tensor_copy(
                    out=v_big.rearrange("p n (h m) -> p n h m", h=HPC)[
                        :, sb, :, 0:64
                    ],
                    in_=pv[:, 0 : HPC * D].rearrange("p (h m) -> p h m", h=HPC),
                )

            # ---- attention ------------------------------------------------
            def emit_attn(qb, p, av2):
                n_k = 4 * (qb + 1)
                sts = {}
                for step in range(n_k + 1):
                    if step < n_k:
                        t = step
                        j = t - 4 * qb  # >= 0 on diagonal blocks
                        w = SB - KT * j if j >= 0 else SB
                        offs = KT * j if j >= 0 else 0
                        sct = scp.tile([128, 2 * SB], f32, tag="sc", name="sct")
                        for i in range(2):
                            hb = 64 * i
                            nc.tensor.matmul(
                                out=sct[:, i * SB : i * SB + w],
                                lhsT=qkbig[hb : hb + 64, 2 + p, t * KT : (t + 1) * KT],
                                rhs=qkbig[
                                    hb : hb + 64, p, qb * SB + offs : (qb + 1) * SB
                                ],
                                start=True,
                                stop=True,
                            )
                        st = stp.tile([128, 2 * SB], f16, tag="st", name="st")
                        sc3 = sct.rearrange("p (i c) -> p i c", i=2)
                        st3 = st.rearrange("p (i c) -> p i c", i=2)
                        nc.scalar.activation(
                            out=st3[:, :, 0:w],
                            in_=sc3[:, :, 0:w],
                            func=mybir.ActivationFunctionType.Exp,
                            scale=0.125,
                        )
                        if j >= 0:
                            nc.vector.tensor_mul(
                                st3[:, :, 0:128],
                                st3[:, :, 0:128],
                                maskc[:, None, :].broadcast_to((128, 2, 128)),
                            )
                        sts[t] = (st, w, offs)
                    if step >= 1:
                        t = step - 1
                        st, w, offs = sts.pop(t)
                        for i in range(2):
                            h = 2 * p + i
                            nc.tensor.matmul(
                                out=av2[0:65, i * SB + offs : (i + 1) * SB],
                                lhsT=v_big[:, t, h * 65 : (h + 1) * 65],
                                rhs=st[:, i * SB : i * SB + w],
                                start=(t == 0),
                                stop=(t == n_k - 1),
                            )

            def emit_attn_pair(qb, p, last=False):
                av2 = accp.tile([128, 2 * SB], f32, tag="acc", name="av2")
                emit_attn(qb, p, av2)
                r2 = nrmp.tile([1, 2 * SB], f32, tag="r2", name="r2")
                zb = nrmp.tile([64, 2 * SB], f32, tag="zb", name="zb")
                if last:
                    # final pair: nothing reuses the psum — normalize it
                    # directly (shorter critical path into the last out-proj)
                    src = av2
                else:
                    # drain psum fast, normalize on the SBUF copy
                    src = nrmp.tile([65, 2 * SB], f16, tag="avS", name="avS")
                    nc.vector.tensor_copy(out=src, in_=av2[0:65, :])
                nc.vector.reciprocal(out=r2, in_=src[64:65, :])
                nc.gpsimd.partition_broadcast(zb, r2)
                for i in range(2):
                    nc.vector.tensor_mul(
                        at8[(p, qb)][64 * i : 64 * i + 64, :],
                        src[0:64, i * SB : (i + 1) * SB],
                        zb[:, i * SB : (i + 1) * SB],
                    )

            # ---- output projection (row-parallel partial), direct f32 DMA -
            out_ap = out_d.ap().rearrange(
                "(qb stl p) (eb c) -> qb stl p eb c", p=128, stl=4, c=512
            )

            def emit_out_proj(qb, eb):
                ot = stp.tile([128, 4, 512], f16, tag="ot", name="ot", bufs=2)
                for stl in range(4):
                    pw = mmp.tile([128, 512], f32, tag="mm", name="pw")
                    for c in range(2):
                        nc.tensor.matmul(
                            out=pw,
                            lhsT=at8[(c, qb)][:, stl * KT : (stl + 1) * KT],
                            rhs=wo[:, c, eb * 512 : (eb + 1) * 512],
                            start=(c == 0),
                            stop=(c == 1),
                        )
                    # alternate the psum drain between DVE and ACT so the
                    # mm-psum rotation never waits on one congested engine
                    if stl % 2 == 0:
                        nc.vector.tensor_copy(out=ot[:, stl, :], in_=pw)
                    else:
                        nc.scalar.copy(out=ot[:, stl, :], in_=pw)
                nc.sync.dma_start(
                    out=out_ap[qb, :, :, eb, :].rearrange("stl p c -> p stl c"),
                    in_=ot,
                )

            # ---- emission schedule ---------------------------------------
            for _ in range(unroll):
                emit_load_head()
                for sb in range(4):
                    emit_qk_proj(sb)
                for sb in range(4):
                    emit_v_proj(sb)
                flush_transposes()
                emit_loads(1)
                emit_qk_proj(4)
                emit_v_proj(4)
                emit_attn_pair(0, 0)
                emit_qk_proj(5)
                emit_v_proj(5)
                flush_transposes()
                emit_loads(2)
                emit_load_wo()
                emit_attn_pair(0, 1)
                emit_qk_proj(6)
                emit_v_proj(6)
                emit_qk_proj(7)
                emit_v_proj(7)
                flush_transposes()
                emit_loads(3)
                emit_qk_proj(8)
                emit_v_proj(8)
                emit_attn_pair(1, 0)
                emit_qk_proj(9)
                emit_v_proj(9)
                flush_transposes()
                emit_attn_pair(1, 1)
                emit_out_proj(0, 0)
                emit_qk_proj(10)
                emit_v_proj(10)
                emit_out_proj(0, 1)
                emit_qk_proj(11)
                emit_v_proj(11)
                flush_transposes()
                emit_qk_proj(12)
                emit_v_proj(12)
                emit_attn_pair(2, 0)
                emit_out_proj(1, 0)
                emit_qk_proj(13)
                emit_v_proj(13)
                flush_transposes()
                emit_attn_pair(2, 1)
                emit_qk_proj(14)
                emit_v_proj(14)
                emit_out_proj(1, 1)
                emit_qk_proj(15)
                emit_v_proj(15)
                flush_transposes()
                emit_out_proj(2, 0)
                emit_attn_pair(3, 0)
                emit_out_proj(2, 1)
                emit_attn_pair(3, 1, last=True)
                emit_out_proj(3, 0)
                emit_out_proj(3, 1)

    nc.compile()
    return nc


def build_in_maps(x, Wq, Wk, Wv, Wo):
    x = np.asarray(x, np.float32)
    Wq = np.asarray(Wq, np.float32)
    Wk = np.asarray(Wk, np.float32)
    Wv = np.asarray(Wv, np.float32)
    Wo = np.asarray(Wo, np.float32)

    # RoPE tables: pos index = sb*128 + partition; pair-frequency index i
    inv = 1.0 / (ROPE_BASE ** (np.arange(0, D, 2, dtype=np.float64) / D))  # [32]
    pos = np.arange(S, dtype=np.float64)
    ang = pos[:, None] * inv[None, :]  # [S, 32]
    cos_t = np.cos(ang).astype(np.float32).reshape(NPB, 128, 32)
    sin_t = np.sin(ang).astype(np.float32).reshape(NPB, 128, 32)
    cosT = np.ascontiguousarray(cos_t.transpose(1, 0, 2)).astype(np.float16)
    sinT = np.concatenate(
        [-sin_t.transpose(1, 0, 2), sin_t.transpose(1, 0, 2)], axis=2
    ).astype(np.float16)  # [128, NPB, 64] = [-sin | +sin]

    # post-exp causal mask for the diagonal 128x128 sub-block: keep (c >= r)
    rr = np.arange(128)[:, None]
    cc = np.arange(128)[None, :]
    maskc = (cc >= rr).astype(np.float16)

    # weight column permutation: even pair-elements then odd (rotate-half)
    perm = np.concatenate([np.arange(0, D, 2), np.arange(1, D, 2)])

    in_maps = []
    for core in range(NCORES):
        b, g = core // HPC, core % HPC
        wqk = np.empty((E, 2 * HPC * D), np.float32)
        for i in range(HPC):
            h = g * HPC + i
            wqk[:, i * D : (i + 1) * D] = Wq[:, h * D : (h + 1) * D][:, perm]
            wqk[:, HPC * D + i * D : HPC * D + (i + 1) * D] = Wk[
                :, h * D : (h + 1) * D
            ][:, perm]
        in_maps.append(
            {
                "xT": np.ascontiguousarray(x[b].T).astype(np.float16),
                "wqk": wqk.astype(np.float16),
                "wv": np.ascontiguousarray(
                    Wv[:, g * HPC * D : (g + 1) * HPC * D]
                ).astype(np.float16),
                "wo": np.ascontiguousarray(
                    Wo[g * HPC * D : (g + 1) * HPC * D, :]
                ).astype(np.float16),
                "cosT": cosT,
                "sinT": sinT,
                "maskc": maskc,
            }
        )
    return in_maps


def gather_output(results):
    outs = [np.asarray(r["out"], np.float32) for r in results]
    return np.stack(
        [outs[0] + outs[1] + outs[2] + outs[3], outs[4] + outs[5] + outs[6] + outs[7]],
        axis=0,
    )


_NC_CACHE = {}


def kernel(x, Wq, Wk, Wv, Wo):
    in_maps = build_in_maps(x, Wq, Wk, Wv, Wo)
    if "nc" not in _NC_CACHE:
        _NC_CACHE["nc"] = build_nc()
    res = run_bass_kernel_spmd(_NC_CACHE["nc"], in_maps, core_ids=list(range(NCORES)))
    return gather_output(res.results)


# revision 39
# speedup vs baseline: 1.0581x; 1.0156x over previous
"""Multi-head causal attention (B=2, S=2048, E=1024, H=16, D=64) on 8 TRN2 cores.

Sharding: core c -> batch b = c // 4, head group g = c % 4 (4 heads each).
Each core computes q/k/v projections + RoPE + causal attention + its rows of
the Wo projection for its (batch, head-group); the host sums the 4 row-parallel
Wo partials per batch.

v2 design (vs v1):
  - Projections produce q/k in [S, D] orientation so RoPE runs on free-dim
    column swaps (3 wide DVE ops per s-block instead of 7 partition-offset
    ones); qT/kT for the scores matmuls are formed by DMA XBAR transposes
    (off the PE critical path).
  - Causal masking is done post-exp with a [128,128] 0/1 multiply on DVE
    (no tri/maskb matmuls on the PE), and the dead q-columns of diagonal
    k-blocks are trimmed from scores/exp/AV.
  - Scores psum is a [128, 1024] two-head tile -> one wide exp per step.
  - av psum is drained by one [128,1024] copy to SBUF; normalization
    (1/Z scale) happens on the SBUF copy, freeing the psum bank early.
  - Out-projection psum is DMA'd to HBM directly as f32 (no copy).
  - Weight/table loads are split per chunk and interleaved in consumption
    order across the two HWDGE queues (SP: x/wqk/tables/out, ACT: wv/wo/
    qkT transposes).
"""

import sys

if "/opt/trn_rl_repo" not in sys.path:
    sys.path.insert(0, "/opt/trn_rl_repo")

import numpy as np

import concourse.bass as bass
import concourse.tile as tile
from concourse import bacc, mybir
from concourse.bass_utils import run_bass_kernel_spmd

B, S, E, H, D = 2, 2048, 1024, 16, 64
HPC = 4  # heads per core
NCORES = 8
SB = 512  # q block width (attention)
NSB = S // SB  # 4
KT = 128  # k tile (partition chunk of the sequence)
PB = 128  # projection s-block
NPB = S // PB  # 16
ECH = E // 128  # 8 contraction chunks for the projections

f32 = mybir.dt.float32
f16 = mybir.dt.float16

ROPE_BASE = 10000.0


def build_nc(unroll=1):
    nc = bacc.Bacc(
        "TRN2", target_bir_lowering=False, debug=False, enable_asserts=False
    )

    xT_d = nc.dram_tensor("xT", [E, S], f16, kind="ExternalInput")
    wqk_d = nc.dram_tensor("wqk", [E, 2 * HPC * D], f16, kind="ExternalInput")
    wv_d = nc.dram_tensor("wv", [E, HPC * D], f16, kind="ExternalInput")
    wo_d = nc.dram_tensor("wo", [HPC * D, E], f16, kind="ExternalInput")
    cos_d = nc.dram_tensor("cosT", [128, NPB, 32], f16, kind="ExternalInput")
    sin_d = nc.dram_tensor("sinT", [128, NPB, 64], f16, kind="ExternalInput")
    mask_d = nc.dram_tensor("maskc", [128, 128], f16, kind="ExternalInput")
    out_d = nc.dram_tensor("out", [S, E], f16, kind="ExternalOutput")

    with tile.TileContext(nc) as tc:
        with (
            tc.tile_pool(name="const", bufs=1) as constp,
            tc.tile_pool(name="rope", bufs=2) as ropep,
            tc.tile_pool(name="st", bufs=4) as stp,
            tc.tile_pool(name="nrm", bufs=2) as nrmp,
            tc.tile_pool(name="mm", bufs=2, space="PSUM") as mmp,
            tc.tile_pool(name="sc", bufs=2, space="PSUM") as scp,
            tc.tile_pool(name="acc", bufs=1, space="PSUM") as accp,
        ):
            # ---- SBUF residents -------------------------------------------
            # x is split over two ping-pong tiles (a: q-blocks 0,2 / b: 1,3)
            # so a later 512-block load never false-serializes against the
            # readers of the other tile (dep tracking is tile-granular).
            xT_ap = xT_d.ap().rearrange("(eo p) s -> p eo s", p=128)
            xTt = [
                constp.tile([128, ECH, S // 2], f16, tag=f"xT{i}", name=f"xT{i}")
                for i in range(2)
            ]

            def xT(sb):
                # lhsT slice [128, 128] for projection s-block sb
                b = sb // 4
                return xTt[b % 2][
                    :, :, (b // 2) * SB + (sb % 4) * PB : (b // 2) * SB + (sb % 4 + 1) * PB
                ]
            wqk = constp.tile([128, ECH, 2 * HPC * D], f16, tag="wqk")
            wqk_ap = wqk_d.ap().rearrange("(eo p) m -> p eo m", p=128)
            wv = constp.tile([128, ECH, HPC * D], f16, tag="wv")
            wo = constp.tile([128, 2, E], f16, tag="wo")
            cosT = constp.tile([128, NPB, 32], f16, tag="cosT")
            sinT = constp.tile([128, NPB, 64], f16, tag="sinT")
            maskc = constp.tile([128, 128], f16, tag="maskc")

            # qkT slots: 0,1 = q pairs (heads 01 / 23), 2,3 = k pairs.
            # Body-scoped tensors are double-buffered by unroll parity so the
            # next body's writes never serialize against this body's readers.
            qkbig2 = [
                constp.tile([128, 4, S], f16, tag=f"qkbig{u}", name=f"qkbig{u}")
                for u in range(2)
            ]
            # v per k-chunk: 4 heads x [v_h (64) | one (1)]
            v_big2 = [
                constp.tile(
                    [128, S // KT, HPC * 65], f16, tag=f"vbig{u}", name=f"vbig{u}"
                )
                for u in range(2)
            ]
            for u in range(2):
                ones_cols = v_big2[u].rearrange("p n (h m) -> p n h m", h=HPC)[
                    :, :, :, 64:65
                ]
                nc.vector.memset(ones_cols, 1.0)

            at8_2 = {}
            for u in range(2):
                for c in range(2):
                    for qb in range(NSB):
                        at8_2[(u, c, qb)] = constp.tile(
                            [128, SB], f16, tag=f"at{u}_{c}_{qb}",
                            name=f"at{u}_{c}_{qb}",
                        )

            # ---- DMA emission helpers -------------------------------------
            def emit_load_head():
                # consumption-ordered: first proj matmul needs wqk[e0]+xT[e0].
                # wv goes on the scalar-engine HWDGE queue (which later
                # carries the qkT XBAR transposes and wo).
                nc.sync.dma_start(out=wqk[:, 0:2, :], in_=wqk_ap[:, 0:2, :])
                nc.sync.dma_start(
                    out=xTt[0][:, 0:2, 0:SB], in_=xT_ap[:, 0:2, 0:SB]
                )
                nc.scalar.dma_start(out=wv, in_=wv_d.ap().rearrange(
                    "(eo p) m -> p eo m", p=128))
                nc.sync.dma_start(out=cosT, in_=cos_d.ap())
                nc.sync.dma_start(out=sinT, in_=sin_d.ap())
                nc.sync.dma_start(
                    out=wqk[:, 2:ECH, :], in_=wqk_ap[:, 2:ECH, :]
                )
                nc.sync.dma_start(
                    out=xTt[0][:, 2:ECH, 0:SB], in_=xT_ap[:, 2:ECH, 0:SB]
                )
                nc.sync.dma_start(out=maskc, in_=mask_d.ap())

            def emit_loads(b):
                cs = slice(b * SB, (b + 1) * SB)
                dst = slice((b // 2) * SB, (b // 2 + 1) * SB)
                nc.sync.dma_start(
                    out=xTt[b % 2][:, :, dst], in_=xT_ap[:, :, cs]
                )

            def emit_load_wo():
                nc.scalar.dma_start(
                    out=wo, in_=wo_d.ap().rearrange("(c p) e -> p c e", p=128)
                )

            # ---- per-s-block projection + rope + transpose ----------------
            pending_t = []
            cur = {}

            def flush_transposes():
                while pending_t:
                    sb, rout = pending_t.pop(0)
                    # one fused XBAR transpose: [128 s, 4*128 f] ->
                    # qkbig[dd, slot, s] (per-slot 128x128 transpose)
                    nc.scalar.dma_start_transpose(
                        out=cur["qk"][:, :, sb * PB : (sb + 1) * PB],
                        in_=rout,
                    )

            def emit_qk_proj(sb):
                cs = slice(sb * PB, (sb + 1) * PB)
                ps = mmp.tile([128, 512], f32, tag="mm", name="ps")
                xts = xT(sb)
                for e in range(ECH):
                    nc.tensor.matmul(
                        out=ps,
                        lhsT=xts[:, e, :],
                        rhs=wqk[:, e, :],
                        start=(e == 0),
                        stop=(e == ECH - 1),
                    )
                # rope: cols = [q(4h) | k(4h)], per head [x1(32) | x2(32)]
                t1 = ropep.tile([128, 512], f16, tag="t1", name="t1")
                t2 = ropep.tile([128, 512], f16, tag="t2", name="t2")
                rout = ropep.tile([128, 512], f16, tag="ro", name="ro", bufs=3)
                ps4 = ps.rearrange("p (g two i) -> p g two i", two=2, i=32)
                t24 = t2.rearrange("p (g two i) -> p g two i", two=2, i=32)
                nc.vector.tensor_mul(
                    t1.rearrange("p (g i) -> p g i", i=32),
                    ps.rearrange("p (g i) -> p g i", i=32),
                    cosT[:, sb, None, :].broadcast_to((128, 16, 32)),
                )
                # o1 part: -x2*sin ; o2 part: +x1*sin
                nc.vector.tensor_mul(
                    t24[:, :, 0:1, :],
                    ps4[:, :, 1:2, :],
                    sinT[:, sb, None, None, 0:32].broadcast_to((128, 8, 1, 32)),
                )
                nc.vector.tensor_mul(
                    t24[:, :, 1:2, :],
                    ps4[:, :, 0:1, :],
                    sinT[:, sb, None, None, 32:64].broadcast_to((128, 8, 1, 32)),
                )
                nc.vector.tensor_add(rout, t1, t2)
                # defer the XBAR transpose so its rope dependency is already
                # satisfied when the scalar queue reaches it (no head-of-line
                # blocking of the exp stream)
                pending_t.append((sb, rout))

            def emit_v_proj(sb):
                pv = mmp.tile([128, 512], f32, tag="mm", name="pv")
                xts = xT(sb)
                for e in range(ECH):
                    nc.tensor.matmul(
                        out=pv[:, 0 : HPC * D],
                        lhsT=xts[:, e, :],
                        rhs=wv[:, e, :],
                        start=(e == 0),
                        stop=(e == ECH - 1),
                    )
                nc.vector.tensor_copy(
                    out=cur["v"].rearrange("p n (h m) -> p n h m", h=HPC)[
                        :, sb, :, 0:64
                    ],
                    in_=pv[:, 0 : HPC * D].rearrange("p (h m) -> p h m", h=HPC),
                )

            # ---- attention ------------------------------------------------
            def emit_attn(qb, p, av2):
                n_k = 4 * (qb + 1)
                sts = {}
                for step in range(n_k + 1):
                    if step < n_k:
                        t = step
                        j = t - 4 * qb  # >= 0 on diagonal blocks
                        w = SB - KT * j if j >= 0 else SB
                        offs = KT * j if j >= 0 else 0
                        sct = scp.tile([128, 2 * SB], f32, tag="sc", name="sct")
                        for i in range(2):
                            hb = 64 * i
                            nc.tensor.matmul(
                                out=sct[:, i * SB : i * SB + w],
                                lhsT=cur["qk"][hb : hb + 64, 2 + p, t * KT : (t + 1) * KT],
                                rhs=cur["qk"][
                                    hb : hb + 64, p, qb * SB + offs : (qb + 1) * SB
                                ],
                                start=True,
                                stop=True,
                            )
                        st = stp.tile([128, 2 * SB], f16, tag="st", name="st")
                        sc3 = sct.rearrange("p (i c) -> p i c", i=2)
                        st3 = st.rearrange("p (i c) -> p i c", i=2)
                        nc.scalar.activation(
                            out=st3[:, :, 0:w],
                            in_=sc3[:, :, 0:w],
                            func=mybir.ActivationFunctionType.Exp,
                            scale=0.125,
                        )
                        if j >= 0:
                            nc.vector.tensor_mul(
                                st3[:, :, 0:128],
                                st3[:, :, 0:128],
                                maskc[:, None, :].broadcast_to((128, 2, 128)),
                            )
                        sts[t] = (st, w, offs)
                    if step >= 1:
                        t = step - 1
                        st, w, offs = sts.pop(t)
                        for i in range(2):
                            h = 2 * p + i
                            nc.tensor.matmul(
                                out=av2[0:65, i * SB + offs : (i + 1) * SB],
                                lhsT=cur["v"][:, t, h * 65 : (h + 1) * 65],
                                rhs=st[:, i * SB : i * SB + w],
                                start=(t == 0),
                                stop=(t == n_k - 1),
                            )

            def emit_attn_pair(qb, p, last=False):
                av2 = accp.tile([128, 2 * SB], f32, tag="acc", name="av2")
                emit_attn(qb, p, av2)
                r2 = nrmp.tile([1, 2 * SB], f32, tag="r2", name="r2")
                zb = nrmp.tile([64, 2 * SB], f32, tag="zb", name="zb")
                if last:
                    # final pair: nothing reuses the psum — normalize it
                    # directly (shorter critical path into the last out-proj)
                    src = av2
                else:
                    # drain psum fast (ACT), normalize the SBUF copy on
                    # gpsimd — keeps the DVE free for rope/copies
                    src = nrmp.tile([65, 2 * SB], f16, tag="avS", name="avS")
                    nc.vector.tensor_copy(out=src, in_=av2[0:65, :])
                nc.vector.reciprocal(out=r2, in_=src[64:65, :])
                nc.gpsimd.partition_broadcast(zb, r2)
                eng = nc.vector if last else nc.gpsimd
                for i in range(2):
                    eng.tensor_mul(
                        cur["at"][(p, qb)][64 * i : 64 * i + 64, :],
                        src[0:64, i * SB : (i + 1) * SB],
                        zb[:, i * SB : (i + 1) * SB],
                    )

            # ---- output projection (row-parallel partial), direct f32 DMA -
            out_ap = out_d.ap().rearrange(
                "(qb stl p) (eb c) -> qb stl p eb c", p=128, stl=4, c=512
            )

            def emit_out_proj(qb, eb):
                ot = stp.tile([128, 4, 512], f16, tag="ot", name="ot", bufs=2)
                for stl in range(4):
                    pw = mmp.tile([128, 512], f32, tag="mm", name="pw")
                    for c in range(2):
                        nc.tensor.matmul(
                            out=pw,
                            lhsT=cur["at"][(c, qb)][:, stl * KT : (stl + 1) * KT],
                            rhs=wo[:, c, eb * 512 : (eb + 1) * 512],
                            start=(c == 0),
                            stop=(c == 1),
                        )
                    nc.vector.tensor_copy(out=ot[:, stl, :], in_=pw)
                nc.sync.dma_start(
                    out=out_ap[qb, :, :, eb, :].rearrange("stl p c -> p stl c"),
                    in_=ot,
                )

            # ---- emission schedule ---------------------------------------
            for it in range(unroll):
                cur["qk"] = qkbig2[it % 2]
                cur["v"] = v_big2[it % 2]
                cur["at"] = {
                    (c, qb): at8_2[(it % 2, c, qb)]
                    for c in range(2)
                    for qb in range(NSB)
                }
                emit_load_head()
                emit_qk_proj(0)
                emit_loads(1)
                for sb in range(1, 4):
                    emit_qk_proj(sb)
                for sb in range(4):
                    emit_v_proj(sb)
                emit_loads(2)
                flush_transposes()
                emit_qk_proj(4)
                emit_v_proj(4)
                emit_attn_pair(0, 0)
                emit_qk_proj(5)
                emit_v_proj(5)
                emit_load_wo()
                emit_attn_pair(0, 1)
                emit_qk_proj(6)
                emit_v_proj(6)
                emit_qk_proj(7)
                emit_v_proj(7)
                emit_loads(3)
                flush_transposes()
                emit_qk_proj(8)
                emit_v_proj(8)
                emit_attn_pair(1, 0)
                emit_qk_proj(9)
                emit_v_proj(9)
                emit_attn_pair(1, 1)
                emit_qk_proj(10)
                emit_v_proj(10)
                emit_out_proj(0, 0)
                emit_qk_proj(11)
                emit_v_proj(11)
                flush_transposes()
                emit_qk_proj(12)
                emit_v_proj(12)
                emit_attn_pair(2, 0)
                emit_qk_proj(13)
                emit_v_proj(13)
                emit_out_proj(0, 1)
                emit_attn_pair(2, 1)
                emit_qk_proj(14)
                emit_v_proj(14)
                emit_out_proj(1, 0)
                emit_qk_proj(15)
                emit_v_proj(15)
                emit_out_proj(1, 1)
                flush_transposes()
                emit_out_proj(2, 0)
                emit_attn_pair(3, 0)
                emit_out_proj(2, 1)
                emit_attn_pair(3, 1, last=True)
                emit_out_proj(3, 0)
                emit_out_proj(3, 1)

    nc.compile()
    return nc


def build_in_maps(x, Wq, Wk, Wv, Wo):
    x = np.asarray(x, np.float32)
    Wq = np.asarray(Wq, np.float32)
    Wk = np.asarray(Wk, np.float32)
    Wv = np.asarray(Wv, np.float32)
    Wo = np.asarray(Wo, np.float32)

    # RoPE tables: pos index = sb*128 + partition; pair-frequency index i
    inv = 1.0 / (ROPE_BASE ** (np.arange(0, D, 2, dtype=np.float64) / D))  # [32]
    pos = np.arange(S, dtype=np.float64)
    ang = pos[:, None] * inv[None, :]  # [S, 32]
    cos_t = np.cos(ang).astype(np.float32).reshape(NPB, 128, 32)
    sin_t = np.sin(ang).astype(np.float32).reshape(NPB, 128, 32)
    cosT = np.ascontiguousarray(cos_t.transpose(1, 0, 2)).astype(np.float16)
    sinT = np.concatenate(
        [-sin_t.transpose(1, 0, 2), sin_t.transpose(1, 0, 2)], axis=2
    ).astype(np.float16)  # [128, NPB, 64] = [-sin | +sin]

    # post-exp causal mask for the diagonal 128x128 sub-block: keep (c >= r)
    rr = np.arange(128)[:, None]
    cc = np.arange(128)[None, :]
    maskc = (cc >= rr).astype(np.float16)

    # weight column permutation: even pair-elements then odd (rotate-half)
    perm = np.concatenate([np.arange(0, D, 2), np.arange(1, D, 2)])

    in_maps = []
    for core in range(NCORES):
        b, g = core // HPC, core % HPC
        wqk = np.empty((E, 2 * HPC * D), np.float32)
        for i in range(HPC):
            h = g * HPC + i
            wqk[:, i * D : (i + 1) * D] = Wq[:, h * D : (h + 1) * D][:, perm]
            wqk[:, HPC * D + i * D : HPC * D + (i + 1) * D] = Wk[
                :, h * D : (h + 1) * D
            ][:, perm]
        in_maps.append(
            {
                "xT": np.ascontiguousarray(x[b].T).astype(np.float16),
                "wqk": wqk.astype(np.float16),
                "wv": np.ascontiguousarray(
                    Wv[:, g * HPC * D : (g + 1) * HPC * D]
                ).astype(np.float16),
                "wo": np.ascontiguousarray(
                    Wo[g * HPC * D : (g + 1) * HPC * D, :]
                ).astype(np.float16),
                "cosT": cosT,
                "sinT": sinT,
                "maskc": maskc,
            }
        )
    return in_maps


def gather_output(results):
    outs = [np.asarray(r["out"], np.float32) for r in results]
    return np.stack(
        [outs[0] + outs[1] + outs[2] + outs[3], outs[4] + outs[5] + outs[6] + outs[7]],
        axis=0,
    )


_NC_CACHE = {}


def kernel(x, Wq, Wk, Wv, Wo):
    in_maps = build_in_maps(x, Wq, Wk, Wv, Wo)
    if "nc" not in _NC_CACHE:
        _NC_CACHE["nc"] = build_nc()
    res = run_bass_kernel_spmd(_NC_CACHE["nc"], in_maps, core_ids=list(range(NCORES)))
    return gather_output(res.results)
